# revision 43
# baseline (speedup 1.0000x reference)
"""Trainium2 Bass kernel for nn_DynamicConv (dense_cnn).

Math: the 12 scaled conv kernels (lengths 3..11, 1->4 channels) collapse by
linearity into one 11-tap FIR per channel; only the last 8 output positions
use masked (shorter) kernel sets, and the last 2 positions of the padded
length are dropped.  The attention MLP that produces the 12 softmax weights
reads only x[0] and is computed on host (0.03% of FLOPs); its result is baked
into the FIR taps passed to the device.

Device strategy (8 cores, batch-parallel, 4 batches/core):
  - x[b] viewed as 128 chunks x F (F = L/128).  PE transposes 128-column
    windows into tiles XT[p, (b,a)] = x[b, a*F + c0 + p].
  - Conv as banded-Toeplitz matmul with the operand roles swapped vs the
    obvious form: stationary = XT (one 128x128 batch block), moving = the
    per-channel Toeplitz matrices side by side, psum = out[a, (c,q)] --
    the conv result lands directly in OUTPUT-natural layout (partition =
    chunk index a), so no back-transpose pass exists.
  - Valid-conv blocking: windows advance by ST=118 so each window's 118
    outputs never need neighbouring windows -> ONE matmul per (b, window)
    (an A+B accumulation pair measures ~2.3x a single matmul on HW).  A
    final window at c0 = F-128 covers the chunk tail with the A+B(+D)
    form: B taps come from an inter-chunk halo tile built by column-
    shifting window 0's transpose (no strided halo DMA), and the masked
    tail correction sd = u^T Dcat is precomputed at load time from the
    last 128 x values, injected into psum partition 127 by a one-hot
    matmul.
  - Conv tiles are copied psum -> SBUF as fp16 into per-batch staging laid
    out [a, c, pos] (3 streams on ScalarE, 1 on DVE); bn_stats over
    512-col ranges of the staging gives per-channel batch stats; bn_aggr
    + a ones-matmul partition-reduce + a tiny [1,8] AllReduce produce
    global mean/var, and scale/shift broadcast to 128 partitions via a
    K=1 ones matmul (no DRAM bounce).
  - Phase 2: fused BN-affine+ReLU (ScalarE, [128, F] per (b,c)) into an
    fp16 stage tile, then row-chunked stores (nsp=2) round-robined over
    the SP HWDGE + gpsimd SWDGE queues only -- the Activation HWDGE would
    contend with the affine ops; one DMA instruction rides one ring
    (~26 GB/s) in isolation, but 4 big concurrent calls across the two
    queues sustain ~190 GB/s effective in-kernel (measured; the isolated
    microbench preferred 9 smaller calls -- trust in-kernel numbers).
  - Output is stored as fp16 (dominant HBM traffic halved; host upcasts
    to f32) and conv matmuls run f32r (TF32-like): worst-case error
    ~8e-4 vs the 5e-3 test gate.  KERNEL_BF16OUT=0 / KERNEL_F32R=0
    restore f32 stores / exact-fp32 matmuls.
"""

import os
import numpy as np

import concourse.bass as bass
import concourse.bacc as bacc
import concourse.tile as tile
from concourse import mybir
from concourse import bass_utils

KS = [3, 3, 3, 5, 5, 7, 7, 7, 9, 9, 11, 11]
B, L, CH = 32, 262144, 4
N_CORES = 8
EPS = 1e-5
MAXK = 11

F32 = mybir.dt.float32
F32R = mybir.dt.float32r
BF16 = mybir.dt.bfloat16
FP16 = mybir.dt.float16


# ---------------------------------------------------------------- host math
def _attention_weights(x0, w1, b1, w2, b2):
    """softmax weights s[12] from batch element 0 (reference uses y[0,0])."""
    n = x0.shape[0]
    pooled = x0.reshape(256, n // 256).mean(axis=1)
    h = np.maximum(w1.astype(np.float32) @ pooled + b1, 0.0).astype(np.float32)
    z = (w2.astype(np.float32) @ h + b2).astype(np.float32)
    z = z - z.max()
    e = np.exp(z)
    return (e / e.sum()).astype(np.float32)


def _fir_taps(s, kernels):
    """K_eff[c, j] and the 8 masked tail variants K_tail[d][c, j]."""
    keff = np.zeros((CH, MAXK), np.float32)
    for i, k in enumerate(kernels):
        keff[:, : KS[i]] += s[i] * k[:, 0, :]
    ktail = np.zeros((8, CH, MAXK), np.float32)
    for d in range(8):
        for i, k in enumerate(kernels):
            if KS[i] <= 10 - d:
                ktail[d, :, : KS[i]] += s[i] * k[:, 0, :]
    return keff, ktail


def _toeplitz_mats(keff, ktail):
    """Banded-Toeplitz conv matrices, boundary corner, and masked tail."""
    A = np.zeros((CH, 128, 128), np.float32)
    Bm = np.zeros((CH, 10, 128), np.float32)
    for c in range(CH):
        for p in range(128):
            for q in range(max(0, p - (MAXK - 1)), p + 1):
                A[c, p, q] = keff[c, p - q]
        for r in range(10):
            for q in range(118 + r, 128):
                j = 128 - q + r
                if j < MAXK:
                    Bm[c, r, q] = keff[c, j]
    # correction for the last 128-column of the last slice: psum there holds
    # A.T@u (B side saw zeros).  D = T - A fixes q in [118,126) to the masked
    # tail value and exactly cancels q in {126,127} (dropped positions).
    D = np.zeros((CH, 128, 128), np.float32)
    for c in range(CH):
        T = np.zeros((128, 128), np.float32)
        for q in range(118, 126):
            d = q - 118
            for j in range(MAXK):
                p = q + j
                if p < 128:
                    T[p, q] = ktail[d, c, j]
        D[c, :, 118:] = T[:, 118:] - A[c, :, 118:]
    return A, Bm, D


ST = 128 - MAXK + 1   # 118: outputs whose 11-tap window fits one slice


def _cat_mats(A, Bm, D):
    """Concatenate per-channel matrices along the q axis: [., CH*128]."""
    Acat = np.concatenate([A[c] for c in range(CH)], axis=1)
    Bcat = np.concatenate([Bm[c] for c in range(CH)], axis=1)
    Dcat = np.concatenate([D[c] for c in range(CH)], axis=1)
    A118 = np.concatenate([A[c][:, 0:ST] for c in range(CH)], axis=1)
    return Acat, Bcat, Dcat, A118


# ---------------------------------------------------------------- device IR
def _build(n_cores, b_loc, length, conv_f32r, bf16out, no_collective=False,
           repeat=1):
    """repeat>1 wraps the whole kernel body in a tc.For_i hardware loop —
    a timing-only variant that executes the identical (idempotent) kernel
    `repeat` times back-to-back on device, so steady-state per-pass time can
    be measured above the ~70-100 ms axon dispatch round-trip noise."""
    F = length // 128            # chunk length per partition row
    NS = F // 128                # number of 128-column slices
    ML = length - 2              # valid output length
    NTOT = float(n_cores * b_loc * ML)
    n_per_part = 128 * NS * b_loc  # bn count per partition/channel

    nc = bacc.Bacc("TRN2", target_bir_lowering=False, debug=False,
                   num_devices=1 if no_collective else n_cores)

    x_d = nc.dram_tensor("x", [b_loc, length], F32, kind="ExternalInput")
    a_d = nc.dram_tensor("amat", [128, CH * 128], F32, kind="ExternalInput")
    b_d = nc.dram_tensor("bmat", [10, CH * 128], F32, kind="ExternalInput")
    d_d = nc.dram_tensor("dmat", [128, CH * 128], F32, kind="ExternalInput")
    a118_d = nc.dram_tensor("a118", [128, CH * ST], F32,
                            kind="ExternalInput")
    gb_d = nc.dram_tensor("gb", [1, 8], F32, kind="ExternalInput")
    odt = FP16 if bf16out else F32
    out_d = nc.dram_tensor("out", [b_loc, CH, ML], odt, kind="ExternalOutput")

    wdt = F32R if conv_f32r else F32
    if os.environ.get("KERNEL_WDT", "") == "fp16" and conv_f32r:
        wdt = FP16
    cdt = FP16 if bf16out else F32

    from contextlib import nullcontext
    with tile.TileContext(nc) as tc:
        with (tc.For_i(0, repeat, 1) if repeat > 1 else nullcontext()), \
             tc.tile_pool(name="singles", bufs=1) as singles, \
             tc.tile_pool(name="xpool", bufs=1) as xpool, \
             tc.tile_pool(name="cpool", bufs=1) as cpool, \
             tc.tile_pool(name="stats", bufs=1) as stats, \
             tc.tile_pool(name="dram", bufs=1, space="DRAM") as dram:

            ident = nc.inline_tensor(np.eye(128, dtype=np.float32),
                                     name="ident")
            ident_sb = singles.tile([128, 128], F32, tag="ident")
            nc.sync.dma_start(out=ident_sb, in_=ident.ap())
            ones = nc.inline_tensor(np.ones((128, 1), np.float32),
                                    name="ones")
            ones_sb = singles.tile([128, 1], F32, tag="ones")
            nc.sync.dma_start(out=ones_sb, in_=ones.ap())
            onesr = nc.inline_tensor(np.ones((1, 128), np.float32),
                                     name="onesr")
            onesr_sb = singles.tile([1, 128], F32, tag="onesr")
            nc.sync.dma_start(out=onesr_sb, in_=onesr.ap())
            e127np = np.zeros((1, 128), np.float32)
            e127np[0, 127] = 1.0
            e127 = nc.inline_tensor(e127np, name="e127")
            e127_f = singles.tile([1, 128], F32, tag="e127f")
            nc.sync.dma_start(out=e127_f, in_=e127.ap())
            e127_sb = singles.tile([1, 128], wdt, tag="e127")
            nc.vector.tensor_copy(e127_sb[:], e127_f[:])

            # conv weight matrices (cast to f32r on device when enabled)
            def _load_w(dten, shape, tag):
                t = singles.tile(shape, F32, tag=tag + "f")
                nc.sync.dma_start(out=t, in_=dten.ap())
                if wdt != F32:
                    tr = singles.tile(shape, wdt, tag=tag)
                    nc.vector.tensor_copy(tr[:], t[:])
                    return tr
                return t

            acat_sb = _load_w(a_d, [128, CH * 128], "acat")
            a118_sb = _load_w(a118_d, [128, CH * ST], "a118")
            bcat_sb = _load_w(b_d, [10, CH * 128], "bcat")
            dcat_sb = _load_w(d_d, [128, CH * 128], "dcat")
            gb_sb = singles.tile([1, 8], F32, tag="gb")
            nc.sync.dma_start(out=gb_sb, in_=gb_d.ap())

            # x tiles [128, F+10] with inter-chunk halo.  Loads round-robin
            # over the three DMA-issue paths (2 HWDGE rings + SWDGE).
            dma_engs = [nc.sync, nc.scalar, nc.gpsimd]
            xeng_names = os.environ.get(
                "KERNEL_XENGS", "sync,gpsimd").split(",")
            xengs = [getattr(nc, e) for e in xeng_names]
            x_tiles = []
            for b in range(b_loc):
                xt = xpool.tile([128, F], F32, tag=f"X{b}")
                xv = x_d.ap()[b].rearrange("(a f) -> a f", f=F)
                nq = int(os.environ.get("KERNEL_XSPLIT", "3"))
                if "noload" in os.environ.get("KERNEL_VARIANT", ""):
                    nc.vector.memset(xt[:], 0.25)
                else:
                    for qi in range(nq):
                        f0, f1 = qi * F // nq, (qi + 1) * F // nq
                        eng = xengs[(b * nq + qi) % len(xengs)]
                        eng.dma_start(out=xt[:, f0:f1], in_=xv[:, f0:f1])
                x_tiles.append(xt)

            # per-batch conv staging [a, c, si*128+q] in 16-bit: phase-2
            # reads per (b,c) are contiguous [128, F] rows
            ctb = [cpool.tile([128, CH, NS * 128], cdt, tag=f"CT{b}",
                              name=f"CT{b}")
                   for b in range(b_loc)]
            # HW bn_stats emits exactly 6 values/partition, so stats run per
            # (batch, channel, 512-col range) over the fp16 staging.
            # bnst[p, c, slot*6] keeps each channel's slots contiguous for a
            # 2-D bn_aggr read.
            NG = (F + 511) // 512
            T_SLOTS = b_loc * NG
            bnst = stats.tile([128, CH, T_SLOTS * 6], F32, tag="bnst",
                              name="bnst")
            if "nostats" in os.environ.get("KERNEL_VARIANT", ""):
                nc.vector.memset(bnst[:], 1.0)  # timing-only variant

            # gpsimd (Pool) cannot access PSUM; Activation takes 3 of 4
            # psum->SBUF copy streams, DVE (which also runs bn_stats) one
            ct_eng_names = os.environ.get(
                "KERNEL_CTENGS", "scalar,scalar,vector,scalar").split(",")
            ct_engs = [getattr(nc, e) for e in ct_eng_names]

            with tc.tile_pool(name="xt", bufs=int(os.environ.get("KERNEL_XT", "4"))) as xtp, \
                 tc.tile_pool(name="ht", bufs=1) as htp, \
                 tc.tile_pool(name="psT", bufs=int(os.environ.get("KERNEL_PST", "2")), space="PSUM") as psT, \
                 tc.tile_pool(name="psC", bufs=int(os.environ.get("KERNEL_PSC", "5")), space="PSUM") as psC, \
                 tc.tile_pool(name="psS", bufs=1, space="PSUM") as psS:

                def make_xt(c0):
                    pst = psT.tile([128, 512], F32, tag="pst")
                    for b in range(b_loc):
                        nc.tensor.transpose(
                            pst[:, b * 128:(b + 1) * 128],
                            x_tiles[b][:, c0:c0 + 128], ident_sb[:])
                    xt4 = xtp.tile([128, 512], wdt, tag="xt4")
                    if os.environ.get("KERNEL_XTENG", "scalar") == "vector":
                        nc.vector.tensor_copy(xt4[:], pst[:])
                    else:
                        nc.scalar.copy(out=xt4[:], in_=pst[:])
                    return xt4

                sd_sbs = []
                for b in range(b_loc):
                    # u = last 128 x values of batch b: one-descriptor row
                    # load, transposed to a column on the PE
                    u_row = stats.tile([1, 128], F32, tag="urow", bufs=4,
                                       name="urow")
                    nc.sync.dma_start(
                        out=u_row[:],
                        in_=x_d.ap()[b, length - 128:length].rearrange(
                            "(a f) -> a f", a=1))
                    psu = psS.tile([128, 8], F32, tag="small", name="psu")
                    nc.tensor.transpose(psu[:, 0:1], u_row[:],
                                        ident_sb[0:1, 0:1])
                    u_sb = stats.tile([128, 1], wdt, tag="usb", bufs=4,
                                      name="usb")
                    nc.vector.tensor_copy(u_sb[:], psu[:, 0:1])
                    ps_d = psS.tile([1, 512], F32, tag="small", name="ps_d")
                    nc.tensor.matmul(ps_d[:], u_sb[:], dcat_sb[:],
                                     start=True, stop=True)
                    sd_sb = stats.tile([1, 512], wdt, tag="sdfix",
                                       bufs=4, name="sdfix")
                    nc.vector.tensor_copy(sd_sb[:], ps_d[:])
                    sd_sbs.append(sd_sb)

                xt_cur = make_xt(0)
                # inter-chunk halo, shifted out of slice 0's transposed tile:
                # chunk a's first-10-positions-of-chunk-(a+1) halo is column
                # a+1 of the slice-0 transpose; chunk 127 has none -> zeros
                ht4 = htp.tile([10, 512], wdt, tag="ht4")
                htz = htp.tile([10, 512], F32, tag="htz")
                nc.vector.memset(htz[:], 0.0)
                nc.vector.tensor_copy(ht4[:], htz[:])
                for b in range(b_loc):
                    nc.vector.tensor_copy(
                        ht4[0:10, b * 128:b * 128 + 127],
                        xt_cur[0:10, b * 128 + 1:(b + 1) * 128])
                # valid-conv blocking: each regular slice reads 128 input
                # positions at c0 = ST*si and produces ST=118 fully-valid
                # outputs (the 11-tap window never leaves the stationary), so
                # conv is ONE matmul per (b, slice) -- an A+B accumulation
                # pair measures ~2.3x the cost of a single matmul on HW.  A
                # final slice at c0 = F-128 covers the chunk tail with the
                # A+B(+D) form; its overlap region recomputes identical
                # values.
                NSL = (F - 128) // ST + 1
                c0f = F - 128

                def fire_stats(b, ks):
                    if "nostats" in os.environ.get("KERNEL_VARIANT", ""):
                        return
                    for k in ks:
                        lo, hi = k * 512, min(k * 512 + 512, F)
                        for c in range(CH):
                            slot = b * NG + k
                            nc.vector.bn_stats(
                                out=bnst[:, c, slot * 6:(slot + 1) * 6],
                                in_=ctb[b][:, c, lo:hi])

                def do_copy(b, pcv, ct_view):
                    eng = ct_engs[b]
                    if hasattr(eng, "tensor_copy"):
                        eng.tensor_copy(ct_view, pcv)
                    else:
                        eng.copy(out=ct_view, in_=pcv)

                trig = {}
                for k in range(NG - 1):
                    trig.setdefault(-(-512 * (k + 1) // ST) - 1, []).append(k)

                for si in range(NSL):
                    c0 = ST * si
                    xt = xt_cur if si == 0 else make_xt(c0)
                    for b in range(b_loc):
                        bs = slice(b * 128, (b + 1) * 128)
                        pc = psC.tile([128, 512], F32, tag="pc")
                        nc.tensor.matmul(pc[:, 0:CH * ST], xt[:, bs],
                                         a118_sb[:], start=True, stop=True)
                        do_copy(b, pc[:, 0:CH * ST].rearrange(
                                    "p (c q) -> p c q", c=CH),
                                ctb[b][:, :, c0:c0 + ST])
                        fire_stats(b, trig.get(si, []))
                # final boundary slice
                xtf = make_xt(c0f)
                for b in range(b_loc):
                    bs = slice(b * 128, (b + 1) * 128)
                    pc = psC.tile([128, 512], F32, tag="pc")
                    nc.tensor.matmul(pc[:], xtf[:, bs], acat_sb[:],
                                     start=True, stop=False)
                    nc.tensor.matmul(pc[:], ht4[0:10, bs], bcat_sb[:],
                                     start=False, stop=False)
                    # masked-tail fix (precomputed sd), re-injected on
                    # partition 127 via one-hot matmul accumulation
                    nc.tensor.matmul(pc[:], e127_sb[:], sd_sbs[b][:],
                                     start=False, stop=True)
                    do_copy(b, pc.rearrange("p (c q) -> p c q", c=CH),
                            ctb[b][:, :, c0f:F])
                    fire_stats(b, [NG - 1])

                # ---- stats finalize + collective
                stats_loc = stats.tile([128, 8], F32, tag="stats_loc")
                mv8 = stats.tile([128, CH, 2], F32, tag="mv8")
                for c in range(CH):
                    nc.vector.bn_aggr(out=mv8[:, c, :], in_=bnst[:, c, :])
                means, vars_ = mv8[:, :, 0], mv8[:, :, 1]
                msq4 = stats.tile([128, 4], F32, tag="msq4")
                nc.vector.tensor_mul(msq4[:], means, means)
                e24 = stats.tile([128, 4], F32, tag="e24")
                nc.vector.tensor_add(e24[:], vars_, msq4[:])
                nc.scalar.mul(out=stats_loc[:, 0:4], in_=means,
                              mul=float(n_per_part))
                nc.scalar.mul(out=stats_loc[:, 4:8], in_=e24[:],
                              mul=float(n_per_part))
                ps_red = psS.tile([1, 8], F32, tag="small", name="ps_red")
                nc.tensor.matmul(ps_red[:], ones_sb[:], stats_loc[:],
                                 start=True, stop=True)
                red_sb = stats.tile([1, 8], F32, tag="red")
                nc.vector.tensor_copy(red_sb[:], ps_red[:])

                cc_in = dram.tile([1, 8], F32)
                cc_out = dram.tile([1, 8], F32)
                nc.gpsimd.dma_start(out=cc_in[:], in_=red_sb[:])
                if no_collective:
                    # timing-model variant: plain DRAM round trip instead of
                    # the AllReduce (TimelineSim is single-core)
                    nc.gpsimd.dma_start(out=cc_out[:], in_=cc_in[:])
                else:
                    nc.gpsimd.collective_compute(
                        "AllReduce", mybir.AluOpType.add,
                        replica_groups=[list(range(n_cores))],
                        ins=[cc_in.opt()], outs=[cc_out.opt()])
                g_sb = stats.tile([1, 8], F32, tag="g")
                nc.gpsimd.dma_start(out=g_sb[:], in_=cc_out[:])

                # scale/shift: a = gamma/sqrt(var+eps), b = beta - mean*a
                mean = stats.tile([1, 4], F32, tag="mean")
                nc.scalar.mul(out=mean[:], in_=g_sb[0:1, 0:4], mul=1.0 / NTOT)
                e2g = stats.tile([1, 4], F32, tag="e2g")
                nc.scalar.mul(out=e2g[:], in_=g_sb[0:1, 4:8], mul=1.0 / NTOT)
                msqg = stats.tile([1, 4], F32, tag="msqg")
                nc.vector.tensor_mul(msqg[:], mean[:], mean[:])
                var = stats.tile([1, 4], F32, tag="var")
                nc.vector.tensor_sub(var[:], e2g[:], msqg[:])
                epst = stats.tile([1, 1], F32, tag="epst")
                nc.vector.memset(epst[:], EPS)
                sd = stats.tile([1, 4], F32, tag="sd")
                nc.scalar.activation(out=sd[:], in_=var[:],
                                     func=mybir.ActivationFunctionType.Sqrt,
                                     bias=epst[:], scale=1.0)
                rstd = stats.tile([1, 4], F32, tag="rstd")
                nc.vector.reciprocal(out=rstd[:], in_=sd[:])
                ab = stats.tile([1, 8], F32, tag="ab")
                nc.vector.tensor_mul(ab[0:1, 0:4], gb_sb[0:1, 0:4], rstd[:])
                tmp = stats.tile([1, 4], F32, tag="tmpb")
                nc.vector.tensor_mul(tmp[:], mean[:], ab[0:1, 0:4])
                nc.vector.tensor_sub(ab[0:1, 4:8], gb_sb[0:1, 4:8], tmp[:])

                # broadcast [1,8] -> [128,8] with a K=1 ones matmul
                ps_bc = psS.tile([128, 8], F32, tag="small", name="ps_bc")
                nc.tensor.matmul(ps_bc[:], onesr_sb[:], ab[:],
                                 start=True, stop=True)
                ab_bc = stats.tile([128, 8], F32, tag="ab_bc")
                nc.vector.tensor_copy(ab_bc[:], ps_bc[:])

            # phase 2: fused BN affine + ReLU + chunked DMA out.
            # Bulk stores avoid the Activation engine by default: its DGE
            # shares the SEQ with the affine ops, delaying its third of the
            # store traffic.
            nsp = int(os.environ.get("KERNEL_OSPLIT", "2"))
            oeng_names = os.environ.get(
                "KERNEL_OENGS", "sync,gpsimd").split(",")
            oengs = [getattr(nc, e) for e in oeng_names]
            variant = os.environ.get("KERNEL_VARIANT", "")
            with tc.tile_pool(name="spool", bufs=4) as spool:
                for b in range(b_loc):
                    for c in range(CH):
                        if "nostore" in variant and not (b == 0 and c == 0):
                            continue
                        st = spool.tile([128, F], cdt, tag="stage")
                        nc.scalar.activation(
                            out=st[:], in_=ctb[b][:, c, :],
                            func=mybir.ActivationFunctionType.Relu,
                            scale=ab_bc[:, c:c + 1],
                            bias=ab_bc[:, 4 + c:5 + c])
                        ov = out_d.ap()[b, c]
                        if "nostore" in variant:
                            nc.sync.dma_start(
                                out=ov[0:F].rearrange("(a f) -> a f", a=1),
                                in_=st[0:1, :])
                            continue
                        for sp_i in range(nsp):
                            p0 = sp_i * 127 // nsp
                            p1 = (sp_i + 1) * 127 // nsp
                            eng = oengs[(b * CH * nsp + c * nsp + sp_i)
                                        % len(oengs)]
                            eng.dma_start(
                                out=ov[0:127 * F].rearrange(
                                    "(a f) -> a f", f=F)[p0:p1, :],
                                in_=st[p0:p1, :])
                        teng = oengs[(b * CH + c + 1) % len(oengs)]
                        teng.dma_start(
                            out=ov[127 * F:ML].rearrange("(a f) -> a f", a=1),
                            in_=st[127:128, 0:F - 2])

    return _finish(nc)


def _finish(nc):
    nc.compile()
    return nc


_CACHE = {}


def _get_nc(n_cores, b_loc, length, conv_f32r, bf16out=None,
            no_collective=False, repeat=1):
    if bf16out is None:
        bf16out = _use_bf16out()
    key = (n_cores, b_loc, length, conv_f32r, bf16out, no_collective, repeat,
           os.environ.get("KERNEL_VARIANT", ""),
           os.environ.get("KERNEL_OSPLIT", "2"),
           os.environ.get("KERNEL_CTENGS", ""),
           os.environ.get("KERNEL_CTLAYOUT", "csq"),
           os.environ.get("KERNEL_XTENG", "scalar"),
           os.environ.get("KERNEL_OENGS", ""),
           os.environ.get("KERNEL_XENGS", ""),
           os.environ.get("KERNEL_WDT", ""),
           os.environ.get("KERNEL_PST", "2"),
           os.environ.get("KERNEL_XT", "4"))
    if key not in _CACHE:
        _CACHE[key] = _build(*key[:7])
    return _CACHE[key]


def _prepare_inputs(x, w1, b1, w2, b2, bn_gamma, bn_beta, kernels,
                    n_cores):
    x = np.ascontiguousarray(np.asarray(x, np.float32))
    bsz, _, length = x.shape
    s = _attention_weights(x[0, 0], np.asarray(w1, np.float32),
                           np.asarray(b1, np.float32),
                           np.asarray(w2, np.float32),
                           np.asarray(b2, np.float32))
    keff, ktail = _fir_taps(s, [np.asarray(k, np.float32) for k in kernels])
    A, Bm, D = _toeplitz_mats(keff, ktail)
    Acat, Bcat, Dcat, A118 = _cat_mats(A, Bm, D)
    gb = np.concatenate([np.asarray(bn_gamma, np.float32),
                         np.asarray(bn_beta, np.float32)])[None, :]
    b_loc = bsz // n_cores
    in_maps = []
    for core in range(n_cores):
        in_maps.append({
            "x": x[core * b_loc:(core + 1) * b_loc, 0, :],
            "amat": Acat, "bmat": Bcat, "dmat": Dcat, "a118": A118,
            "gb": gb,
        })
    return in_maps, b_loc, length


def _use_f32r():
    """f32r (TF32-like 1 cyc/row PE path) is the default: ~2e-4 max rel err
    against the 5e-3 tolerance, and 4x faster conv matmuls."""
    return os.environ.get("KERNEL_F32R", "1") == "1"


def _use_bf16out():
    """bf16 output staging/stores (dominant HBM traffic halved; ~2e-3 max
    quantization vs the 5e-3 tolerance).  Host upcasts to f32."""
    return os.environ.get("KERNEL_BF16OUT", "1") == "1"


def run(inputs, n_cores=N_CORES, conv_f32r=None, trace=False):
    if conv_f32r is None:
        conv_f32r = _use_f32r()
    kernels = [inputs[f"k{i}"] for i in range(len(KS))]
    in_maps, b_loc, length = _prepare_inputs(
        inputs["x"], inputs["w1"], inputs["b1"], inputs["w2"], inputs["b2"],
        inputs["bn_gamma"], inputs["bn_beta"], kernels, n_cores)
    nc = _get_nc(n_cores, b_loc, length, conv_f32r)
    try:
        res = bass_utils.run_bass_kernel_spmd(
            nc, in_maps, core_ids=list(range(n_cores)), trace=trace)
    except ModuleNotFoundError:
        # no axon NTFF profiling hook in this container
        res = bass_utils.run_bass_kernel_spmd(
            nc, in_maps, core_ids=list(range(n_cores)), trace=False)
    out = np.concatenate(
        [np.asarray(res.results[c]["out"]).astype(np.float32)
         for c in range(n_cores)], axis=0)
    return out, res


def kernel(**inputs):
    out, _ = run(inputs)
    return out


# revision 44
# speedup vs baseline: 1.0108x; 1.0108x over previous
"""Trainium2 Bass kernel for nn_DynamicConv (dense_cnn).

Math: the 12 scaled conv kernels (lengths 3..11, 1->4 channels) collapse by
linearity into one 11-tap FIR per channel; only the last 8 output positions
use masked (shorter) kernel sets, and the last 2 positions of the padded
length are dropped.  The attention MLP that produces the 12 softmax weights
reads only x[0] and is computed on host (0.03% of FLOPs); its result is baked
into the FIR taps passed to the device.

Device strategy (8 cores, batch-parallel, 4 batches/core):
  - x[b] viewed as 128 chunks x F (F = L/128).  PE transposes 128-column
    windows into tiles XT[p, (b,a)] = x[b, a*F + c0 + p].
  - Conv as banded-Toeplitz matmul with the operand roles swapped vs the
    obvious form: stationary = XT (one 128x128 batch block), moving = the
    per-channel Toeplitz matrices side by side, psum = out[a, (c,q)] --
    the conv result lands directly in OUTPUT-natural layout (partition =
    chunk index a), so no back-transpose pass exists.
  - Valid-conv blocking: windows advance by ST=118 so each window's 118
    outputs never need neighbouring windows -> ONE matmul per (b, window)
    (an A+B accumulation pair measures ~2.3x a single matmul on HW).  A
    final window at c0 = F-128 covers the chunk tail with the A+B(+D)
    form: B taps come from an inter-chunk halo tile built by column-
    shifting window 0's transpose (no strided halo DMA), and the masked
    tail correction sd = u^T Dcat is precomputed at load time from the
    last 128 x values, injected into psum partition 127 by a one-hot
    matmul.
  - Conv tiles are copied psum -> SBUF as fp16 into per-batch staging laid
    out [a, c, pos] (3 streams on ScalarE, 1 on DVE); bn_stats over
    512-col ranges of the staging gives per-channel batch stats; bn_aggr
    + a ones-matmul partition-reduce + a tiny [1,8] AllReduce produce
    global mean/var, and scale/shift broadcast to 128 partitions via a
    K=1 ones matmul (no DRAM bounce).
  - Phase 2: fused BN-affine+ReLU (ScalarE, [128, F] per (b,c)) into an
    fp16 stage tile, then row-chunked stores (nsp=2) round-robined over
    the SP HWDGE + gpsimd SWDGE queues only -- the Activation HWDGE would
    contend with the affine ops; one DMA instruction rides one ring
    (~26 GB/s) in isolation, but 4 big concurrent calls across the two
    queues sustain ~190 GB/s effective in-kernel (measured; the isolated
    microbench preferred 9 smaller calls -- trust in-kernel numbers).
  - Output is stored as fp16 (dominant HBM traffic halved; host upcasts
    to f32) and conv matmuls run f32r (TF32-like): worst-case error
    ~8e-4 vs the 5e-3 test gate.  KERNEL_BF16OUT=0 / KERNEL_F32R=0
    restore f32 stores / exact-fp32 matmuls.
"""

import os
import numpy as np

import concourse.bass as bass
import concourse.bacc as bacc
import concourse.tile as tile
from concourse import mybir
from concourse import bass_utils

KS = [3, 3, 3, 5, 5, 7, 7, 7, 9, 9, 11, 11]
B, L, CH = 32, 262144, 4
N_CORES = 8
EPS = 1e-5
MAXK = 11

F32 = mybir.dt.float32
F32R = mybir.dt.float32r
BF16 = mybir.dt.bfloat16
FP16 = mybir.dt.float16


# ---------------------------------------------------------------- host math
def _attention_weights(x0, w1, b1, w2, b2):
    """softmax weights s[12] from batch element 0 (reference uses y[0,0])."""
    n = x0.shape[0]
    pooled = x0.reshape(256, n // 256).mean(axis=1)
    h = np.maximum(w1.astype(np.float32) @ pooled + b1, 0.0).astype(np.float32)
    z = (w2.astype(np.float32) @ h + b2).astype(np.float32)
    z = z - z.max()
    e = np.exp(z)
    return (e / e.sum()).astype(np.float32)


def _fir_taps(s, kernels):
    """K_eff[c, j] and the 8 masked tail variants K_tail[d][c, j]."""
    keff = np.zeros((CH, MAXK), np.float32)
    for i, k in enumerate(kernels):
        keff[:, : KS[i]] += s[i] * k[:, 0, :]
    ktail = np.zeros((8, CH, MAXK), np.float32)
    for d in range(8):
        for i, k in enumerate(kernels):
            if KS[i] <= 10 - d:
                ktail[d, :, : KS[i]] += s[i] * k[:, 0, :]
    return keff, ktail


def _toeplitz_mats(keff, ktail):
    """Banded-Toeplitz conv matrices, boundary corner, and masked tail."""
    A = np.zeros((CH, 128, 128), np.float32)
    Bm = np.zeros((CH, 10, 128), np.float32)
    for c in range(CH):
        for p in range(128):
            for q in range(max(0, p - (MAXK - 1)), p + 1):
                A[c, p, q] = keff[c, p - q]
        for r in range(10):
            for q in range(118 + r, 128):
                j = 128 - q + r
                if j < MAXK:
                    Bm[c, r, q] = keff[c, j]
    # correction for the last 128-column of the last slice: psum there holds
    # A.T@u (B side saw zeros).  D = T - A fixes q in [118,126) to the masked
    # tail value and exactly cancels q in {126,127} (dropped positions).
    D = np.zeros((CH, 128, 128), np.float32)
    for c in range(CH):
        T = np.zeros((128, 128), np.float32)
        for q in range(118, 126):
            d = q - 118
            for j in range(MAXK):
                p = q + j
                if p < 128:
                    T[p, q] = ktail[d, c, j]
        D[c, :, 118:] = T[:, 118:] - A[c, :, 118:]
    return A, Bm, D


ST = 128 - MAXK + 1   # 118: outputs whose 11-tap window fits one slice


def _cat_mats(A, Bm, D):
    """Concatenate per-channel matrices along the q axis: [., CH*128]."""
    Acat = np.concatenate([A[c] for c in range(CH)], axis=1)
    Bcat = np.concatenate([Bm[c] for c in range(CH)], axis=1)
    Dcat = np.concatenate([D[c] for c in range(CH)], axis=1)
    A118 = np.concatenate([A[c][:, 0:ST] for c in range(CH)], axis=1)
    return Acat, Bcat, Dcat, A118


# ---------------------------------------------------------------- device IR
def _build(n_cores, b_loc, length, conv_f32r, bf16out, no_collective=False,
           repeat=1):
    """repeat>1 wraps the whole kernel body in a tc.For_i hardware loop —
    a timing-only variant that executes the identical (idempotent) kernel
    `repeat` times back-to-back on device, so steady-state per-pass time can
    be measured above the ~70-100 ms axon dispatch round-trip noise."""
    F = length // 128            # chunk length per partition row
    NS = F // 128                # number of 128-column slices
    ML = length - 2              # valid output length
    NTOT = float(n_cores * b_loc * ML)
    n_per_part = 128 * NS * b_loc  # bn count per partition/channel

    nc = bacc.Bacc("TRN2", target_bir_lowering=False, debug=False,
                   num_devices=1 if no_collective else n_cores)

    x_d = nc.dram_tensor("x", [b_loc, length], F32, kind="ExternalInput")
    a_d = nc.dram_tensor("amat", [128, CH * 128], F32, kind="ExternalInput")
    b_d = nc.dram_tensor("bmat", [10, CH * 128], F32, kind="ExternalInput")
    d_d = nc.dram_tensor("dmat", [128, CH * 128], F32, kind="ExternalInput")
    a118_d = nc.dram_tensor("a118", [128, CH * ST], F32,
                            kind="ExternalInput")
    gb_d = nc.dram_tensor("gb", [1, 8], F32, kind="ExternalInput")
    odt = FP16 if bf16out else F32
    out_d = nc.dram_tensor("out", [b_loc, CH, ML], odt, kind="ExternalOutput")

    wdt = F32R if conv_f32r else F32
    if os.environ.get("KERNEL_WDT", "") == "fp16" and conv_f32r:
        wdt = FP16
    cdt = FP16 if bf16out else F32

    from contextlib import nullcontext
    with tile.TileContext(nc) as tc:
        with (tc.For_i(0, repeat, 1) if repeat > 1 else nullcontext()), \
             tc.tile_pool(name="singles", bufs=1) as singles, \
             tc.tile_pool(name="xpool", bufs=1) as xpool, \
             tc.tile_pool(name="cpool", bufs=1) as cpool, \
             tc.tile_pool(name="stats", bufs=1) as stats, \
             tc.tile_pool(name="dram", bufs=1, space="DRAM") as dram:

            ident = nc.inline_tensor(np.eye(128, dtype=np.float32),
                                     name="ident")
            ident_sb = singles.tile([128, 128], F32, tag="ident")
            nc.sync.dma_start(out=ident_sb, in_=ident.ap())
            ones = nc.inline_tensor(np.ones((128, 1), np.float32),
                                    name="ones")
            ones_sb = singles.tile([128, 1], F32, tag="ones")
            nc.sync.dma_start(out=ones_sb, in_=ones.ap())
            onesr = nc.inline_tensor(np.ones((1, 128), np.float32),
                                     name="onesr")
            onesr_sb = singles.tile([1, 128], F32, tag="onesr")
            nc.sync.dma_start(out=onesr_sb, in_=onesr.ap())
            e127np = np.zeros((1, 128), np.float32)
            e127np[0, 127] = 1.0
            e127 = nc.inline_tensor(e127np, name="e127")
            e127_f = singles.tile([1, 128], F32, tag="e127f")
            nc.sync.dma_start(out=e127_f, in_=e127.ap())
            e127_sb = singles.tile([1, 128], wdt, tag="e127")
            nc.vector.tensor_copy(e127_sb[:], e127_f[:])

            # conv weight matrices (cast to f32r on device when enabled)
            def _load_w(dten, shape, tag):
                t = singles.tile(shape, F32, tag=tag + "f")
                nc.sync.dma_start(out=t, in_=dten.ap())
                if wdt != F32:
                    tr = singles.tile(shape, wdt, tag=tag)
                    nc.vector.tensor_copy(tr[:], t[:])
                    return tr
                return t

            acat_sb = _load_w(a_d, [128, CH * 128], "acat")
            a118_sb = _load_w(a118_d, [128, CH * ST], "a118")
            bcat_sb = _load_w(b_d, [10, CH * 128], "bcat")
            dcat_sb = _load_w(d_d, [128, CH * 128], "dcat")
            gb_sb = singles.tile([1, 8], F32, tag="gb")
            nc.sync.dma_start(out=gb_sb, in_=gb_d.ap())

            # x tiles [128, F+10] with inter-chunk halo.  Loads round-robin
            # over the three DMA-issue paths (2 HWDGE rings + SWDGE).
            dma_engs = [nc.sync, nc.scalar, nc.gpsimd]
            xeng_names = os.environ.get(
                "KERNEL_XENGS", "sync,gpsimd").split(",")
            xengs = [getattr(nc, e) for e in xeng_names]
            x_tiles = []
            for b in range(b_loc):
                xt = xpool.tile([128, F], F32, tag=f"X{b}")
                xv = x_d.ap()[b].rearrange("(a f) -> a f", f=F)
                nq = int(os.environ.get("KERNEL_XSPLIT", "3"))
                if "noload" in os.environ.get("KERNEL_VARIANT", ""):
                    nc.vector.memset(xt[:], 0.25)
                else:
                    for qi in range(nq):
                        f0, f1 = qi * F // nq, (qi + 1) * F // nq
                        eng = xengs[(b * nq + qi) % len(xengs)]
                        eng.dma_start(out=xt[:, f0:f1], in_=xv[:, f0:f1])
                x_tiles.append(xt)

            # per-batch conv staging [a, c, si*128+q] in 16-bit: phase-2
            # reads per (b,c) are contiguous [128, F] rows
            ctb = [cpool.tile([128, CH, NS * 128], cdt, tag=f"CT{b}",
                              name=f"CT{b}")
                   for b in range(b_loc)]
            # HW bn_stats emits exactly 6 values/partition, so stats run per
            # (batch, channel, 512-col range) over the fp16 staging.
            # bnst[p, c, slot*6] keeps each channel's slots contiguous for a
            # 2-D bn_aggr read.
            RS = int(os.environ.get("KERNEL_RS", "512"))  # stats range cols
            NG = (F + RS - 1) // RS
            T_SLOTS = b_loc * NG
            bnst = stats.tile([128, CH, T_SLOTS * 6], F32, tag="bnst",
                              name="bnst")
            if "nostats" in os.environ.get("KERNEL_VARIANT", ""):
                nc.vector.memset(bnst[:], 1.0)  # timing-only variant

            # gpsimd (Pool) cannot access PSUM; Activation takes 3 of 4
            # psum->SBUF copy streams, DVE (which also runs bn_stats) one
            ct_eng_names = os.environ.get(
                "KERNEL_CTENGS", "scalar,scalar,vector,scalar").split(",")
            ct_engs = [getattr(nc, e) for e in ct_eng_names]

            with tc.tile_pool(name="xt", bufs=int(os.environ.get("KERNEL_XT", "4"))) as xtp, \
                 tc.tile_pool(name="ht", bufs=1) as htp, \
                 tc.tile_pool(name="psT", bufs=int(os.environ.get("KERNEL_PST", "2")), space="PSUM") as psT, \
                 tc.tile_pool(name="psC", bufs=int(os.environ.get("KERNEL_PSC", "5")), space="PSUM") as psC, \
                 tc.tile_pool(name="psS", bufs=1, space="PSUM") as psS:

                def make_xt(c0):
                    pst = psT.tile([128, 512], F32, tag="pst")
                    for b in range(b_loc):
                        nc.tensor.transpose(
                            pst[:, b * 128:(b + 1) * 128],
                            x_tiles[b][:, c0:c0 + 128], ident_sb[:])
                    xt4 = xtp.tile([128, 512], wdt, tag="xt4")
                    if os.environ.get("KERNEL_XTENG", "scalar") == "vector":
                        nc.vector.tensor_copy(xt4[:], pst[:])
                    else:
                        nc.scalar.copy(out=xt4[:], in_=pst[:])
                    return xt4

                sd_sbs = []
                for b in range(b_loc):
                    # u = last 128 x values of batch b: one-descriptor row
                    # load, transposed to a column on the PE
                    u_row = stats.tile([1, 128], F32, tag="urow", bufs=4,
                                       name="urow")
                    nc.sync.dma_start(
                        out=u_row[:],
                        in_=x_d.ap()[b, length - 128:length].rearrange(
                            "(a f) -> a f", a=1))
                    psu = psS.tile([128, 8], F32, tag="small", name="psu")
                    nc.tensor.transpose(psu[:, 0:1], u_row[:],
                                        ident_sb[0:1, 0:1])
                    u_sb = stats.tile([128, 1], wdt, tag="usb", bufs=4,
                                      name="usb")
                    nc.vector.tensor_copy(u_sb[:], psu[:, 0:1])
                    ps_d = psS.tile([1, 512], F32, tag="small", name="ps_d")
                    nc.tensor.matmul(ps_d[:], u_sb[:], dcat_sb[:],
                                     start=True, stop=True)
                    sd_sb = stats.tile([1, 512], wdt, tag="sdfix",
                                       bufs=4, name="sdfix")
                    nc.vector.tensor_copy(sd_sb[:], ps_d[:])
                    sd_sbs.append(sd_sb)

                xt_cur = make_xt(0)
                # inter-chunk halo, shifted out of slice 0's transposed tile:
                # chunk a's first-10-positions-of-chunk-(a+1) halo is column
                # a+1 of the slice-0 transpose; chunk 127 has none -> zeros
                ht4 = htp.tile([10, 512], wdt, tag="ht4")
                htz = htp.tile([10, 512], F32, tag="htz")
                nc.vector.memset(htz[:], 0.0)
                nc.vector.tensor_copy(ht4[:], htz[:])
                for b in range(b_loc):
                    nc.vector.tensor_copy(
                        ht4[0:10, b * 128:b * 128 + 127],
                        xt_cur[0:10, b * 128 + 1:(b + 1) * 128])
                # valid-conv blocking: each regular slice reads 128 input
                # positions at c0 = ST*si and produces ST=118 fully-valid
                # outputs (the 11-tap window never leaves the stationary), so
                # conv is ONE matmul per (b, slice) -- an A+B accumulation
                # pair measures ~2.3x the cost of a single matmul on HW.  A
                # final slice at c0 = F-128 covers the chunk tail with the
                # A+B(+D) form; its overlap region recomputes identical
                # values.
                NSL = (F - 128) // ST + 1
                c0f = F - 128

                def fire_stats(b, ks):
                    if "nostats" in os.environ.get("KERNEL_VARIANT", ""):
                        return
                    for k in ks:
                        lo, hi = k * RS, min(k * RS + RS, F)
                        for c in range(CH):
                            slot = b * NG + k
                            nc.vector.bn_stats(
                                out=bnst[:, c, slot * 6:(slot + 1) * 6],
                                in_=ctb[b][:, c, lo:hi])

                def do_copy(b, pcv, ct_view):
                    eng = ct_engs[b]
                    if hasattr(eng, "tensor_copy"):
                        eng.tensor_copy(ct_view, pcv)
                    else:
                        eng.copy(out=ct_view, in_=pcv)

                trig = {}
                for k in range(NG - 1):
                    trig.setdefault(-(-RS * (k + 1) // ST) - 1, []).append(k)

                for si in range(NSL):
                    c0 = ST * si
                    xt = xt_cur if si == 0 else make_xt(c0)
                    for b in range(b_loc):
                        bs = slice(b * 128, (b + 1) * 128)
                        pc = psC.tile([128, 512], F32, tag="pc")
                        nc.tensor.matmul(pc[:, 0:CH * ST], xt[:, bs],
                                         a118_sb[:], start=True, stop=True)
                        do_copy(b, pc[:, 0:CH * ST].rearrange(
                                    "p (c q) -> p c q", c=CH),
                                ctb[b][:, :, c0:c0 + ST])
                        fire_stats(b, trig.get(si, []))
                # final boundary slice
                xtf = make_xt(c0f)
                for b in range(b_loc):
                    bs = slice(b * 128, (b + 1) * 128)
                    pc = psC.tile([128, 512], F32, tag="pc")
                    nc.tensor.matmul(pc[:], xtf[:, bs], acat_sb[:],
                                     start=True, stop=False)
                    nc.tensor.matmul(pc[:], ht4[0:10, bs], bcat_sb[:],
                                     start=False, stop=False)
                    # masked-tail fix (precomputed sd), re-injected on
                    # partition 127 via one-hot matmul accumulation
                    nc.tensor.matmul(pc[:], e127_sb[:], sd_sbs[b][:],
                                     start=False, stop=True)
                    do_copy(b, pc.rearrange("p (c q) -> p c q", c=CH),
                            ctb[b][:, :, c0f:F])
                    fire_stats(b, [NG - 1])

                # ---- stats finalize + collective
                stats_loc = stats.tile([128, 8], F32, tag="stats_loc")
                mv8 = stats.tile([128, CH, 2], F32, tag="mv8")
                for c in range(CH):
                    nc.vector.bn_aggr(out=mv8[:, c, :], in_=bnst[:, c, :])
                means, vars_ = mv8[:, :, 0], mv8[:, :, 1]
                msq4 = stats.tile([128, 4], F32, tag="msq4")
                nc.vector.tensor_mul(msq4[:], means, means)
                e24 = stats.tile([128, 4], F32, tag="e24")
                nc.vector.tensor_add(e24[:], vars_, msq4[:])
                nc.scalar.mul(out=stats_loc[:, 0:4], in_=means,
                              mul=float(n_per_part))
                nc.scalar.mul(out=stats_loc[:, 4:8], in_=e24[:],
                              mul=float(n_per_part))
                ps_red = psS.tile([1, 8], F32, tag="small", name="ps_red")
                nc.tensor.matmul(ps_red[:], ones_sb[:], stats_loc[:],
                                 start=True, stop=True)
                red_sb = stats.tile([1, 8], F32, tag="red")
                nc.vector.tensor_copy(red_sb[:], ps_red[:])

                cc_in = dram.tile([1, 8], F32)
                cc_out = dram.tile([1, 8], F32)
                nc.gpsimd.dma_start(out=cc_in[:], in_=red_sb[:])
                if no_collective:
                    # timing-model variant: plain DRAM round trip instead of
                    # the AllReduce (TimelineSim is single-core)
                    nc.gpsimd.dma_start(out=cc_out[:], in_=cc_in[:])
                else:
                    nc.gpsimd.collective_compute(
                        "AllReduce", mybir.AluOpType.add,
                        replica_groups=[list(range(n_cores))],
                        ins=[cc_in.opt()], outs=[cc_out.opt()])
                g_sb = stats.tile([1, 8], F32, tag="g")
                nc.gpsimd.dma_start(out=g_sb[:], in_=cc_out[:])

                # scale/shift: a = gamma/sqrt(var+eps), b = beta - mean*a
                mean = stats.tile([1, 4], F32, tag="mean")
                nc.scalar.mul(out=mean[:], in_=g_sb[0:1, 0:4], mul=1.0 / NTOT)
                e2g = stats.tile([1, 4], F32, tag="e2g")
                nc.scalar.mul(out=e2g[:], in_=g_sb[0:1, 4:8], mul=1.0 / NTOT)
                msqg = stats.tile([1, 4], F32, tag="msqg")
                nc.vector.tensor_mul(msqg[:], mean[:], mean[:])
                var = stats.tile([1, 4], F32, tag="var")
                nc.vector.tensor_sub(var[:], e2g[:], msqg[:])
                epst = stats.tile([1, 1], F32, tag="epst")
                nc.vector.memset(epst[:], EPS)
                sd = stats.tile([1, 4], F32, tag="sd")
                nc.scalar.activation(out=sd[:], in_=var[:],
                                     func=mybir.ActivationFunctionType.Sqrt,
                                     bias=epst[:], scale=1.0)
                rstd = stats.tile([1, 4], F32, tag="rstd")
                nc.vector.reciprocal(out=rstd[:], in_=sd[:])
                ab = stats.tile([1, 8], F32, tag="ab")
                nc.vector.tensor_mul(ab[0:1, 0:4], gb_sb[0:1, 0:4], rstd[:])
                tmp = stats.tile([1, 4], F32, tag="tmpb")
                nc.vector.tensor_mul(tmp[:], mean[:], ab[0:1, 0:4])
                nc.vector.tensor_sub(ab[0:1, 4:8], gb_sb[0:1, 4:8], tmp[:])

                # broadcast [1,8] -> [128,8] with a K=1 ones matmul
                ps_bc = psS.tile([128, 8], F32, tag="small", name="ps_bc")
                nc.tensor.matmul(ps_bc[:], onesr_sb[:], ab[:],
                                 start=True, stop=True)
                ab_bc = stats.tile([128, 8], F32, tag="ab_bc")
                nc.vector.tensor_copy(ab_bc[:], ps_bc[:])

            # phase 2: fused BN affine + ReLU + chunked DMA out.
            # Bulk stores avoid the Activation engine by default: its DGE
            # shares the SEQ with the affine ops, delaying its third of the
            # store traffic.
            nsp = int(os.environ.get("KERNEL_OSPLIT", "2"))
            oeng_names = os.environ.get(
                "KERNEL_OENGS", "sync,gpsimd").split(",")
            oengs = [getattr(nc, e) for e in oeng_names]
            variant = os.environ.get("KERNEL_VARIANT", "")
            with tc.tile_pool(name="spool", bufs=4) as spool:
                for b in range(b_loc):
                    for c in range(CH):
                        if "nostore" in variant and not (b == 0 and c == 0):
                            continue
                        st = spool.tile([128, F], cdt, tag="stage")
                        nc.scalar.activation(
                            out=st[:], in_=ctb[b][:, c, :],
                            func=mybir.ActivationFunctionType.Relu,
                            scale=ab_bc[:, c:c + 1],
                            bias=ab_bc[:, 4 + c:5 + c])
                        ov = out_d.ap()[b, c]
                        if "nostore" in variant:
                            nc.sync.dma_start(
                                out=ov[0:F].rearrange("(a f) -> a f", a=1),
                                in_=st[0:1, :])
                            continue
                        for sp_i in range(nsp):
                            p0 = sp_i * 127 // nsp
                            p1 = (sp_i + 1) * 127 // nsp
                            eng = oengs[(b * CH * nsp + c * nsp + sp_i)
                                        % len(oengs)]
                            eng.dma_start(
                                out=ov[0:127 * F].rearrange(
                                    "(a f) -> a f", f=F)[p0:p1, :],
                                in_=st[p0:p1, :])
                        teng = oengs[(b * CH + c + 1) % len(oengs)]
                        teng.dma_start(
                            out=ov[127 * F:ML].rearrange("(a f) -> a f", a=1),
                            in_=st[127:128, 0:F - 2])

    return _finish(nc)


def _finish(nc):
    nc.compile()
    return nc


_CACHE = {}


def _get_nc(n_cores, b_loc, length, conv_f32r, bf16out=None,
            no_collective=False, repeat=1):
    if bf16out is None:
        bf16out = _use_bf16out()
    key = (n_cores, b_loc, length, conv_f32r, bf16out, no_collective, repeat,
           os.environ.get("KERNEL_VARIANT", ""),
           os.environ.get("KERNEL_OSPLIT", "2"),
           os.environ.get("KERNEL_CTENGS", ""),
           os.environ.get("KERNEL_CTLAYOUT", "csq"),
           os.environ.get("KERNEL_XTENG", "scalar"),
           os.environ.get("KERNEL_OENGS", ""),
           os.environ.get("KERNEL_XENGS", ""),
           os.environ.get("KERNEL_WDT", ""),
           os.environ.get("KERNEL_RS", "512"),
           os.environ.get("KERNEL_PST", "2"),
           os.environ.get("KERNEL_XT", "4"))
    if key not in _CACHE:
        _CACHE[key] = _build(*key[:7])
    return _CACHE[key]


def _prepare_inputs(x, w1, b1, w2, b2, bn_gamma, bn_beta, kernels,
                    n_cores):
    x = np.ascontiguousarray(np.asarray(x, np.float32))
    bsz, _, length = x.shape
    s = _attention_weights(x[0, 0], np.asarray(w1, np.float32),
                           np.asarray(b1, np.float32),
                           np.asarray(w2, np.float32),
                           np.asarray(b2, np.float32))
    keff, ktail = _fir_taps(s, [np.asarray(k, np.float32) for k in kernels])
    A, Bm, D = _toeplitz_mats(keff, ktail)
    Acat, Bcat, Dcat, A118 = _cat_mats(A, Bm, D)
    gb = np.concatenate([np.asarray(bn_gamma, np.float32),
                         np.asarray(bn_beta, np.float32)])[None, :]
    b_loc = bsz // n_cores
    in_maps = []
    for core in range(n_cores):
        in_maps.append({
            "x": x[core * b_loc:(core + 1) * b_loc, 0, :],
            "amat": Acat, "bmat": Bcat, "dmat": Dcat, "a118": A118,
            "gb": gb,
        })
    return in_maps, b_loc, length


def _use_f32r():
    """f32r (TF32-like 1 cyc/row PE path) is the default: ~2e-4 max rel err
    against the 5e-3 tolerance, and 4x faster conv matmuls."""
    return os.environ.get("KERNEL_F32R", "1") == "1"


def _use_bf16out():
    """bf16 output staging/stores (dominant HBM traffic halved; ~2e-3 max
    quantization vs the 5e-3 tolerance).  Host upcasts to f32."""
    return os.environ.get("KERNEL_BF16OUT", "1") == "1"


def run(inputs, n_cores=N_CORES, conv_f32r=None, trace=False):
    if conv_f32r is None:
        conv_f32r = _use_f32r()
    kernels = [inputs[f"k{i}"] for i in range(len(KS))]
    in_maps, b_loc, length = _prepare_inputs(
        inputs["x"], inputs["w1"], inputs["b1"], inputs["w2"], inputs["b2"],
        inputs["bn_gamma"], inputs["bn_beta"], kernels, n_cores)
    nc = _get_nc(n_cores, b_loc, length, conv_f32r)
    try:
        res = bass_utils.run_bass_kernel_spmd(
            nc, in_maps, core_ids=list(range(n_cores)), trace=trace)
    except ModuleNotFoundError:
        # no axon NTFF profiling hook in this container
        res = bass_utils.run_bass_kernel_spmd(
            nc, in_maps, core_ids=list(range(n_cores)), trace=False)
    out = np.concatenate(
        [np.asarray(res.results[c]["out"]).astype(np.float32)
         for c in range(n_cores)], axis=0)
    return out, res


def kernel(**inputs):
    out, _ = run(inputs)
    return out
